# revision 13
# baseline (speedup 1.0000x reference)
"""3-layer GATv2 (PyG GATv2Conv semantics) on 8 Trainium2 NeuronCores.

Distribution: nodes sharded 12500/core (padded to 12544=98*128 rows in DRAM
tables); edges (incl. self-loops) partitioned by dst core, grouped into
128-edge slots per 128-dst-node block. Per layer:
  phase A: [xl|xr] = h @ [Wl|Wr] for local nodes (PE, lhsT = feature-major
           hT kept resident in SBUF). xr half stays in SBUF (xlr_all);
           xl rows go to DRAM and AllGather -> every core holds all rows
           (layers 2,3 only).
  phase B: per superblock of SB blocks:
           xl[src]: layer 1 from host-pregathered x[src] (xeT) via PE
           matmul with W1l; layers 2/3 via 4 dma_gather calls (one per
           src%4 class, each pinned to its own SWDGE queue so all four
           Q7 core pairs generate descriptors in parallel).
           xr[dst]: dst rows are local; a node-major indicator S_nm
           (built from a K=1 row-replication matmul of dstrel + is_equal
           against the partition index) broadcasts SBUF-resident xr rows
           to edge lanes via PE matmuls - no dst gather at all.
           z = xl+xr, lrelu, score = reduce(z*a), w = exp(score),
           per-block indicator matmul S_em.T @ [w*xl | w] accumulates
           weighted sums + denominators in PSUM. Divide, bias, ELU, and
           write hT feature-major straight back into SBUF for the next
           layer. Layer 3: divide, head-mean, bias into an SBUF staging
           buffer; single f32 DMA out at the end.
"""
import sys
sys.path.insert(0, "/opt/trn_rl_repo")
import numpy as np
import ml_dtypes

N = 100000
E = 800000
NCORES = 8
SHARD = N // NCORES        # 12500
P = 128
NBLK = (SHARD + P - 1) // P  # 98
SHARDP = NBLK * P            # 12544 (padded rows in DRAM tables)
SB = 3                       # node blocks per superblock
NSB = (NBLK + SB - 1) // SB  # 33
FIN = 64
H = 4
C1, C3 = 16, 32
F1 = H * C1                # 64
F3 = H * C3                # 128
NEG_SLOPE = 0.2
NCLS = 4

BF16 = ml_dtypes.bfloat16

_cache = {}


class Meta:
    pass


def _slot_layout(cnts, per_class):
    """cnts: [NCORES, NBLK(, NCLS)] edge counts. Returns group counts per
    (block(,class)) = ceil(max over cores / P), min 1, plus per-sb slot maps.
    Slot order within a superblock: class-major, block-minor (l23) or
    block-major (l1)."""
    mx = cnts.max(axis=0)
    G = np.maximum(1, -(-mx // P))      # [NBLK(, NCLS)]
    sb_blocks = [list(range(s * SB, min(NBLK, (s + 1) * SB))) for s in range(NSB)]
    blk_slots = [[] for _ in range(NBLK)]
    sb_g, sb_cls_off = [], []
    for s, bs in enumerate(sb_blocks):
        slot = 0
        offs = [0]
        if per_class:
            for r in range(NCLS):
                for b in bs:
                    for _ in range(G[b, r]):
                        blk_slots[b].append(slot)
                        slot += 1
                offs.append(slot)
        else:
            for b in bs:
                for _ in range(G[b]):
                    blk_slots[b].append(slot)
                    slot += 1
            offs.append(slot)
        sb_cls_off.append(offs)
        sb_g.append(slot)
    goff = np.concatenate([[0], np.cumsum(sb_g)]).astype(int)
    return G, sb_blocks, blk_slots, sb_g, sb_cls_off, [int(v) for v in goff]


def _preprocess(edge_index):
    src = np.concatenate([edge_index[0], np.arange(N, dtype=np.int32)])
    dst = np.concatenate([edge_index[1], np.arange(N, dtype=np.int32)])
    order = np.argsort(dst, kind="stable")
    src_s = src[order].astype(np.int64)
    dst_s = dst[order].astype(np.int64)

    core = dst_s // SHARD
    blk = (dst_s - core * SHARD) // P
    m = Meta()

    # ---------------- layer 2/3 structure: per (core, block, src%4) ----------
    cls = src_s % NCLS
    key = (core * NBLK + blk) * NCLS + cls
    cnt23 = np.bincount(key, minlength=NCORES * NBLK * NCLS) \
        .reshape(NCORES, NBLK, NCLS)
    (m.G23, m.sb_blocks, m.blk23_slots, m.sb23_g, m.sb23_off,
     m.goff23) = _slot_layout(cnt23, True)
    m.Gtot23 = m.goff23[-1]
    m.SBG23 = max(m.sb23_g)

    # ---------------- layer 1 structure: per (core, block), no classes -------
    cnt1 = np.bincount(core * NBLK + blk, minlength=NCORES * NBLK) \
        .reshape(NCORES, NBLK)
    (m.G1, _, m.blk1_slots, m.sb1_g, _, m.goff1) = _slot_layout(cnt1, False)
    m.Gtot1 = m.goff1[-1]
    m.SBG1 = max(m.sb1_g)

    # ---------------- per-core edge placement --------------------------------
    starts = np.concatenate([[0], np.cumsum(
        np.bincount(core * NBLK + blk, minlength=NCORES * NBLK))])
    # per (core, block): sorted edge ranges (already dst-sorted within)
    m.rel1 = np.full((NCORES, m.Gtot1 * P), -1.0, np.float32)
    m.se1 = np.full((NCORES, m.Gtot1 * P), -1, np.int64)   # src node or -1
    m.rel23 = np.full((NCORES, m.Gtot23 * P), -1.0, np.float32)
    idx23 = np.zeros((NCORES, m.Gtot23 * P), np.int16)     # padded row // 4
    for c in range(NCORES):
        for b in range(NBLK):
            i = c * NBLK + b
            s0, e0 = starts[i], starts[i + 1]
            sr, dr = src_s[s0:e0], dst_s[s0:e0]
            rel = (dr - c * SHARD - b * P).astype(np.float32)
            # layer 1: block-major slots
            sb = b // SB
            sl0 = m.goff1[sb] + m.blk1_slots[b][0]
            o = sl0 * P
            m.se1[c, o:o + len(sr)] = sr
            m.rel1[c, o:o + len(sr)] = rel
            # layers 2/3: class-major slots
            cl = sr % NCLS
            for r in range(NCLS):
                sel = cl == r
                srr, relr = sr[sel], rel[sel]
                # slot range for (b, r) inside superblock sb
                before = sum(m.G23[bb, r] for bb in m.sb_blocks[sb] if bb < b)
                slr0 = m.sb23_off[sb][r] + before
                o = (m.goff23[sb] + slr0) * P
                prow = (srr // SHARD) * SHARDP + (srr % SHARD)  # padded row
                idx23[c, o:o + len(srr)] = (prow // NCLS).astype(np.int16)
                m.rel23[c, o:o + len(srr)] = relr
    assert (m.rel1[m.se1 >= 0] >= 0).all()

    # srcw packed per queue: class r data on partitions [32r, 32r+32)
    # (both 16-partition halves of Q7 core pair r get a full copy).
    # free offsets per (class, sb) recorded in m.q_off.
    m.q_off = np.zeros((NCLS, NSB + 1), np.int64)
    for r in range(NCLS):
        tot = 0
        for s in range(NSB):
            m.q_off[r, s] = tot
            tot += (m.sb23_off[s][r + 1] - m.sb23_off[s][r]) * P // 16
        m.q_off[r, NSB] = tot
    WQ = int(m.q_off[:, NSB].max())
    m.WQ = WQ
    srcw = np.zeros((NCORES, P, WQ), np.int16)
    for c in range(NCORES):
        for r in range(NCLS):
            for s in range(NSB):
                g0 = m.sb23_off[s][r] + m.goff23[s]
                g1 = m.sb23_off[s][r + 1] + m.goff23[s]
                flat = idx23[c, g0 * P:g1 * P]
                if len(flat) == 0:
                    continue
                w = flat.reshape(-1, 16).T          # [16, n/16]
                o = m.q_off[r, s]
                srcw[c, 32 * r:32 * r + 16, o:o + w.shape[1]] = w
                srcw[c, 32 * r + 16:32 * r + 32, o:o + w.shape[1]] = w
    m.srcw = srcw

    m.rel1_pm = m.rel1.reshape(NCORES, m.Gtot1, P).transpose(0, 2, 1) \
        .astype(BF16).copy()
    m.rel23_pm = m.rel23.reshape(NCORES, m.Gtot23, P).transpose(0, 2, 1) \
        .astype(BF16).copy()

    # free-major dstrel per superblock row: [NSB, SBG*P]
    def free_major(rel, goff, SBG):
        out = np.full((NCORES, NSB, SBG * P), -1.0, np.float32)
        for s in range(NSB):
            g0, g1 = goff[s], goff[s + 1]
            out[:, s, :(g1 - g0) * P] = rel[:, g0 * P:g1 * P]
        return out.astype(BF16)

    m.relT1 = free_major(m.rel1, m.goff1, m.SBG1)
    m.relT23 = free_major(m.rel23, m.goff23, m.SBG23)
    return m


def _build_program(m):
    import concourse.bass as bass
    import concourse.bacc as bacc
    import concourse.tile as tile
    from concourse import mybir, library_config

    bf16, f32, i16 = mybir.dt.bfloat16, mybir.dt.float32, mybir.dt.int16
    AF = mybir.ActivationFunctionType
    OP = mybir.AluOpType
    X = mybir.AxisListType.X

    nc = bacc.Bacc("TRN2", target_bir_lowering=False, num_swdge_queues=4)

    xT_d = nc.dram_tensor("xT", [FIN, SHARD], bf16, kind="ExternalInput")
    xeT_d = nc.dram_tensor("xeT", [FIN, m.Gtot1 * P], bf16, kind="ExternalInput")
    srcw_d = nc.dram_tensor("srcw", [P, m.WQ], i16, kind="ExternalInput")
    rel1_d = nc.dram_tensor("rel1", [P, m.Gtot1], bf16, kind="ExternalInput")
    rel23_d = nc.dram_tensor("rel23", [P, m.Gtot23], bf16, kind="ExternalInput")
    iota_d = nc.dram_tensor("iota", [P, P], bf16, kind="ExternalInput")
    ident_d = nc.dram_tensor("ident", [P, P], bf16, kind="ExternalInput")
    W_d = [nc.dram_tensor(f"W{l}", [FIN, 2 * (F1 if l < 3 else F3)], bf16,
                          kind="ExternalInput") for l in (1, 2, 3)]
    arep_d = [nc.dram_tensor(f"arep{l}", [P, F1 if l < 3 else F3], bf16,
                             kind="ExternalInput") for l in (1, 2, 3)]
    brep_d = [nc.dram_tensor("brep1", [P, F1], bf16, kind="ExternalInput"),
              nc.dram_tensor("brep2", [P, F1], bf16, kind="ExternalInput"),
              nc.dram_tensor("brep3", [P, C3], f32, kind="ExternalInput")]
    out_d = nc.dram_tensor("out_shard", [SHARDP, C3], f32, kind="ExternalOutput")

    def internal(name, shape, dt, shared=False):
        return nc.dram_tensor(name, shape, dt, kind="Internal",
                              addr_space="Shared" if shared else "Local")

    # xl tables for layers 2,3: local shard rows + AllGather output
    xsh_d = [internal(f"xsh{l}", [SHARDP, 2 * F1 if l == 2 else F3], bf16)
             for l in (2, 3)]
    cc_d = [internal(f"cc{l}", [NCORES * SHARDP, 2 * F1 if l == 2 else F3],
                     bf16, shared=True) for l in (2, 3)]

    RG = [list(range(NCORES))]
    CH = 8   # slots per psum chunk (xr broadcast / l1 xl matmul)
    CHT = 8  # slots per S_nm transpose chunk

    with tile.TileContext(nc) as tc:
        nc.gpsimd.load_library(library_config.mlp)
        with tc.tile_pool(name="const", bufs=1) as cpool, \
             tc.tile_pool(name="gat", bufs=2) as gpool, \
             tc.tile_pool(name="wrk", bufs=1) as wpool, \
             tc.tile_pool(name="wl2", bufs=2) as w2pool, \
             tc.tile_pool(name="tail", bufs=2) as tpool, \
             tc.tile_pool(name="psA", bufs=1, space="PSUM") as ppA, \
             tc.tile_pool(name="psD", bufs=1, space="PSUM") as ppD, \
             tc.tile_pool(name="psX", bufs=2, space="PSUM") as ppX, \
             tc.tile_pool(name="psB", bufs=1, space="PSUM") as ppB, \
             tc.tile_pool(name="psT", bufs=1, space="PSUM") as ppT:

            # ---------------- constants ----------------
            srcw = cpool.tile([P, m.WQ], i16)
            rel1 = cpool.tile([P, m.Gtot1], bf16)
            rel23 = cpool.tile([P, m.Gtot23], bf16)
            iota = cpool.tile([P, P], bf16)
            ident = cpool.tile([P, P], bf16)
            for t, d in [(srcw, srcw_d), (rel1, rel1_d), (rel23, rel23_d),
                         (iota, iota_d), (ident, ident_d)]:
                nc.sync.dma_start(t[:], d[:])
            W_sb, arep_sb, brep_sb = [], [], []
            for li in range(3):
                Fl = F1 if li < 2 else F3
                w = cpool.tile([FIN, 2 * Fl], bf16, tag=f"W{li}")
                nc.sync.dma_start(w[:], W_d[li][:])
                W_sb.append(w)
                a = cpool.tile([P, Fl], bf16, tag=f"arep{li}")
                nc.sync.dma_start(a[:], arep_d[li][:])
                arep_sb.append(a)
                b = cpool.tile([P, F1 if li < 2 else C3], bf16 if li < 2 else f32,
                               tag=f"brep{li}")
                nc.sync.dma_start(b[:], brep_d[li][:])
                brep_sb.append(b)

            # persistent SBUF state
            hT = cpool.tile([FIN, SHARD], bf16)       # current layer input, fmajor
            nc.sync.dma_start(hT[:], xT_d[:])
            xlr = cpool.tile([P, NBLK, 2 * F3], bf16)  # [xl|xr] rows per block
            outb = cpool.tile([P, NBLK, C3], f32)      # layer-3 staging

            for li in range(3):
                l3 = (li == 2)
                Fl = F3 if l3 else F1
                Cl = C3 if l3 else C1
                FE = 2 * Fl
                if li == 0:
                    SBG, sb_g, goff = m.SBG1, m.sb1_g, m.goff1
                    blk_slots, rel = m.blk1_slots, rel1
                else:
                    SBG, sb_g, goff = m.SBG23, m.sb23_g, m.goff23
                    blk_slots, rel = m.blk23_slots, rel23

                # ---- phase A: [xl|xr] for local nodes ----
                for t in range(NBLK):
                    n0 = t * P
                    mm = min(P, SHARD - n0)
                    psA = ppA.tile([P, 2 * F3], f32, tag="psA", space="PSUM")
                    nc.tensor.matmul(psA[:mm, :FE], lhsT=hT[:, n0:n0 + mm],
                                     rhs=W_sb[li][:], start=True, stop=True)
                    if mm < P:
                        # zero pad rows (32-aligned partition base; real rows
                        # are rewritten by the copy below): keeps the DMA'd
                        # table clean and avoids 0*garbage NaNs in the
                        # xr-broadcast matmul
                        nc.vector.memset(xlr[(mm // 32) * 32:, t, :FE], 0.0)
                    nc.scalar.copy(xlr[:mm, t, :FE], psA[:mm, :FE])

                if li > 0:
                    TW = FE if li == 1 else F3   # table row width (l3: xl only)
                    nc.sync.dma_start(
                        xsh_d[li - 1][:].rearrange("(t p) f -> p t f", p=P),
                        xlr[:, :, :TW])
                    nc.gpsimd.collective_compute(
                        "AllGather", mybir.AluOpType.bypass, replica_groups=RG,
                        ins=[xsh_d[li - 1][:]], outs=[cc_d[li - 1][:]])
                    src_tabs = [cc_d[li - 1][r::NCLS, :] for r in range(NCLS)]
                    GELEM = TW
                    GSTEP = NCLS * TW

                # ---- phase B ----
                for s in range(NSB):
                    SG = sb_g[s]
                    go = goff[s]
                    # l1/l2: [:, :, :64]=xl, [64:128]=z scratch; l3: full 128=xl3
                    xg = gpool.tile([P, SBG, 128], bf16, tag="xg")
                    if li == 0:
                        # host-pregathered x[src]: xl = xeT.T @ W1l per slot
                        xeTt = gpool.tile([FIN, m.SBG1 * P], bf16, tag="xeTt")
                        nc.sync.dma_start(xeTt[:, :SG * P],
                                          xeT_d[:, go * P:(go + SG) * P])
                        for c0 in range(0, SG, CH):
                            c1 = min(SG, c0 + CH)
                            psX = ppX.tile([P, CH, F3], f32, tag="psx", space="PSUM")
                            for j, sl in enumerate(range(c0, c1)):
                                nc.tensor.matmul(
                                    psX[:, j, :Fl], lhsT=xeTt[:, sl * P:(sl + 1) * P],
                                    rhs=W_sb[0][:, :Fl], start=True, stop=True)
                            nc.scalar.copy(xg[:, c0:c1, :Fl], psX[:, :c1 - c0, :Fl])
                    else:
                        for r in range(NCLS):
                            o0 = m.sb23_off[s][r]
                            o1 = m.sb23_off[s][r + 1]
                            nr = (o1 - o0) * P
                            if nr == 0:
                                continue
                            nc.gpsimd.dma_gather(
                                out_ap=xg[:, o0:o1, :GELEM], in_ap=src_tabs[r],
                                idxs_ap=srcw[:, m.q_off[r, s]:m.q_off[r, s] + nr // 16],
                                num_idxs=nr, num_idxs_reg=nr, elem_size=GELEM,
                                elem_step=GSTEP, single_packet=False, queue_num=r)

                    # S_em: edge-major indicator  [e, slot, node]
                    S = w2pool.tile([P, SBG, P], bf16, tag="S")
                    nc.vector.tensor_tensor(
                        out=S[:, :SG, :],
                        in0=iota[:].rearrange("p (g n) -> p g n", g=1)
                            .to_broadcast([P, SG, P]),
                        in1=rel[:, go:go + SG].to_broadcast([P, SG, P]),
                        op=OP.is_equal)
                    # S_nm: node-major indicator = per-slot PE transpose of S_em
                    Snm = wpool.tile([P, SBG, P], bf16, tag="Snm")
                    for c0 in range(0, SG, CHT):
                        c1 = min(SG, c0 + CHT)
                        psD = ppD.tile([P, CHT * P], bf16, tag="psD", space="PSUM")
                        for j, sl in enumerate(range(c0, c1)):
                            nc.tensor.transpose(psD[:, j * P:(j + 1) * P],
                                                S[:, sl, :], ident[:])
                        nc.scalar.copy(
                            Snm[:, c0:c1, :],
                            psD[:, :(c1 - c0) * P]
                                .rearrange("p (g n) -> p g n", g=c1 - c0))

                    # xr[dst] broadcast to edge lanes + z = xl + xr
                    xgv = xg[:, :SG, :Fl]
                    if l3:
                        zvt = wpool.tile([P, SBG, F3], bf16, tag="zv")
                        zv = zvt[:, :SG, :]
                    else:
                        zv = xg[:, :SG, Fl:FE]   # dead half of gathered rows
                    xrcol = Fl                   # xr column base in xlr rows
                    sblk = m.slot_block[li > 0]
                    for c0 in range(0, SG, CH):
                        c1 = min(SG, c0 + CH)
                        psX = ppX.tile([P, CH, F3], f32, tag="psx", space="PSUM")
                        for j, sl in enumerate(range(c0, c1)):
                            nc.tensor.matmul(
                                psX[:, j, :Fl], lhsT=Snm[:, sl, :],
                                rhs=xlr[:, sblk[go + sl], xrcol:xrcol + Fl],
                                start=True, stop=True)
                        nc.vector.tensor_tensor(
                            out=zv[:, c0:c1, :], in0=xgv[:, c0:c1, :],
                            in1=psX[:, :c1 - c0, :Fl], op=OP.add)

                    # lrelu, score, softmax numerator, wlhs = [w*xl | w]
                    nc.vector.scalar_tensor_tensor(
                        out=zv, in0=zv, scalar=NEG_SLOPE, in1=zv,
                        op0=OP.mult, op1=OP.max)
                    nc.vector.tensor_tensor(
                        out=zv, in0=zv,
                        in1=arep_sb[li][:].rearrange("p (g f) -> p g f", g=1)
                            .to_broadcast([P, SG, Fl]),
                        op=OP.mult)
                    score = w2pool.tile([P, SBG * H], f32, tag="score")
                    nc.vector.tensor_reduce(
                        out=score[:, :SG * H],
                        in_=zv.rearrange("p g (h c) -> p g h c", h=H),
                        axis=X, op=OP.add)
                    wlhs = w2pool.tile([P, SBG, F3 + H], bf16, tag="wlhs")
                    nc.scalar.activation(
                        wlhs[:, :SG, :Fl].rearrange("p g (h c) -> p g h c", h=H),
                        score[:, :SG * H].rearrange("p (g h) -> p g h", g=SG)
                            .to_broadcast([P, SG, H, Cl]),
                        AF.Exp)
                    nc.scalar.activation(
                        wlhs[:, :SG, Fl:Fl + H],
                        score[:, :SG * H].rearrange("p (g h) -> p g h", g=SG),
                        AF.Exp)
                    nc.vector.tensor_tensor(out=wlhs[:, :SG, :Fl], in0=xgv,
                                            in1=wlhs[:, :SG, :Fl], op=OP.mult)

                    # per-block aggregation + epilogue
                    for b in m.sb_blocks[s]:
                        n0 = b * P
                        mm = min(P, SHARD - n0)
                        slots = blk_slots[b]
                        psB = ppB.tile([P, F3 + H], f32, tag="psB", space="PSUM")
                        for i, sl in enumerate(slots):
                            nc.tensor.matmul(psB[:, :Fl + H], lhsT=S[:, sl, :],
                                             rhs=wlhs[:, sl, :Fl + H],
                                             start=(i == 0),
                                             stop=(i == len(slots) - 1))
                        rec = tpool.tile([P, H], f32, tag="rec")
                        nc.vector.reciprocal(rec[:], psB[:, Fl:Fl + H])
                        if not l3:
                            hb = tpool.tile([P, F1], bf16, tag="hb")
                            nc.vector.tensor_tensor(
                                out=hb[:].rearrange("p (h c) -> p h c", h=H),
                                in0=psB[:, :Fl].rearrange("p (h c) -> p h c", h=H),
                                in1=rec[:, :, None].to_broadcast([P, H, Cl]),
                                op=OP.mult)
                            nc.vector.tensor_tensor(out=hb[:], in0=hb[:],
                                                    in1=brep_sb[li][:], op=OP.add)
                            rp = tpool.tile([P, F1], bf16, tag="rp")
                            nc.scalar.activation(rp[:], hb[:], AF.Relu)
                            xm = tpool.tile([P, F1], bf16, tag="xm")
                            nc.vector.tensor_scalar_min(out=xm[:], in0=hb[:],
                                                        scalar1=0.0)
                            ex = tpool.tile([P, F1], f32, tag="ex")
                            nc.scalar.activation(ex[:], xm[:], AF.Exp)
                            ho = tpool.tile([P, F1], bf16, tag="ho")
                            nc.vector.scalar_tensor_tensor(
                                out=ho[:], in0=ex[:], scalar=-1.0, in1=rp[:],
                                op0=OP.add, op1=OP.add)
                            psT = ppT.tile([F1, P], bf16, tag="psT", space="PSUM")
                            nc.tensor.transpose(psT[:, :mm], ho[:mm, :],
                                                ident[:mm, :mm])
                            nc.scalar.copy(hT[:, n0:n0 + mm], psT[:, :mm])
                        else:
                            o3 = tpool.tile([P, F3], f32, tag="o3")
                            nc.vector.tensor_tensor(
                                out=o3[:].rearrange("p (h c) -> p h c", h=H),
                                in0=psB[:, :Fl].rearrange("p (h c) -> p h c", h=H),
                                in1=rec[:, :, None].to_broadcast([P, H, Cl]),
                                op=OP.mult)
                            ms = tpool.tile([P, C3], f32, tag="ms")
                            nc.vector.tensor_reduce(
                                out=ms[:],
                                in_=o3[:].rearrange("p (h c) -> p c h", h=H),
                                axis=X, op=OP.add)
                            nc.vector.scalar_tensor_tensor(
                                out=outb[:, b, :], in0=ms[:], scalar=0.25,
                                in1=brep_sb[2][:], op0=OP.mult, op1=OP.add)

            nc.sync.dma_start(out_d[:].rearrange("(t p) c -> p t c", p=P),
                              outb[:])

    nc.compile()
    return nc


def _prep_inputs(x, edge_index, Ws, atts, bs):
    m = _preprocess(edge_index)
    # slot -> block maps (device program indexes xlr by the slot's block)
    sb1 = np.zeros(m.Gtot1, np.int64)
    sb23 = np.zeros(m.Gtot23, np.int64)
    for b in range(NBLK):
        s = b // SB
        for sl in m.blk1_slots[b]:
            sb1[m.goff1[s] + sl] = b
        for sl in m.blk23_slots[b]:
            sb23[m.goff23[s] + sl] = b
    m.slot_block = {False: sb1, True: sb23}

    ident = np.eye(P, dtype=np.float32).astype(BF16)
    iota = np.broadcast_to(np.arange(P, dtype=np.float32), (P, P)).astype(BF16).copy()
    common = {"ident": ident, "iota": iota}
    for li, ((Wl, Wr), a, b) in enumerate(zip(Ws, atts, bs)):
        Fl = Wl.shape[1]
        common[f"W{li + 1}"] = np.concatenate([Wl, Wr], axis=1).astype(BF16)
        a_flat = np.asarray(a).reshape(Fl).astype(np.float32)
        common[f"arep{li + 1}"] = np.broadcast_to(a_flat, (P, Fl)).astype(BF16).copy()
        bdt = np.float32 if li == 2 else BF16
        common[f"brep{li + 1}"] = np.broadcast_to(
            np.asarray(b, np.float32), (P, len(np.asarray(b)))).astype(bdt).copy()

    x_bf = x.astype(BF16)
    in_maps = []
    for c in range(NCORES):
        d = dict(common)
        d["xT"] = x_bf[c * SHARD:(c + 1) * SHARD].T.copy()
        se = m.se1[c]
        xe = x_bf[np.maximum(se, 0)]
        xe[se < 0] = 0
        d["xeT"] = xe.T.copy()
        d["srcw"] = m.srcw[c]
        d["rel1"] = m.rel1_pm[c]
        d["rel23"] = m.rel23_pm[c]
        in_maps.append(d)
    return in_maps, m


def kernel(x, edge_index, W1l, W1r, a1, b1, W2l, W2r, a2, b2, W3l, W3r, a3, b3,
           _trace=False):
    from concourse.bass_utils import run_bass_kernel_spmd

    x = np.asarray(x, dtype=np.float32)
    edge_index = np.asarray(edge_index, dtype=np.int32)
    in_maps, m = _prep_inputs(
        x, edge_index,
        [(np.asarray(W1l), np.asarray(W1r)), (np.asarray(W2l), np.asarray(W2r)),
         (np.asarray(W3l), np.asarray(W3r))],
        [a1, a2, a3], [b1, b2, b3])

    key = (m.Gtot1, m.Gtot23, tuple(m.sb1_g), tuple(m.sb23_g),
           tuple(tuple(o) for o in m.sb23_off))
    if key not in _cache:
        _cache.clear()
        _cache[key] = _build_program(m)
    nc = _cache[key]

    res = run_bass_kernel_spmd(nc, in_maps, core_ids=list(range(NCORES)),
                               trace=_trace)
    out = np.concatenate([res.results[c]["out_shard"][:SHARD]
                          for c in range(NCORES)], axis=0)
    kernel._last_result = res
    return out
